# revision 1
# baseline (speedup 1.0000x reference)
"""Embedding lookup (KVEmbedding) on 8 TRN2 NeuronCores.

Strategy: the 256 MB table fits in HBM, so replicate it on every core and
shard the indices along batch (2048 rows/core). Each core runs a 3-stage
DMA pipeline over 25 tiles of 16384 lookups:
  1. HWDGE load of a [128, K] int32 index tile (contiguous, 64 KB)
  2. SWDGE indirect gather table[idx] -> SBUF [128, K*64] (16384 rows/instr)
  3. HWDGE store of the [128, K*64] f32 tile to the output (contiguous 4 MB)
No collectives needed; output shards concatenate on host.
"""

import numpy as np

BATCH, HIST = 16384, 200
VOCAB, D = 1_000_000, 64
NCORES = 8
ROWS_PER_CORE = BATCH // NCORES          # 2048
FLAT = ROWS_PER_CORE * HIST              # 409600 lookups per core
P = 128                                  # SBUF partitions
K = 128                                  # indices per partition per tile
TILE_ROWS = P * K                        # 16384
NTILES = FLAT // TILE_ROWS               # 25

_built = None


def _build(flat=FLAT, vocab=VOCAB, d=D, k=K, bufs=4):
    """Raw-Bass 2-queue pipeline.

    Tile's auto-semaphores emit 2 embedded waits on steady-state gathers
    (WAW on the slot's previous gather + WAR on the freeing store), but the
    DMA ISA struct holds only one sync-wait -> codegen ICE. Raw Bass keeps
    every DMA at zero embedded waits (standalone sequencer waits) and one
    sem update.
      gpsimd (Pool/SWDGE):  indirect gathers  table[idx] -> SBUF slot i%bufs
      sync   (SP/HWDGE):    idx preload, then contiguous stores slot -> out
    """
    from contextlib import ExitStack

    import concourse.bass as bass
    import concourse.mybir as mybir

    ntiles = flat // (P * k)
    assert ntiles * P * k == flat

    nc = bass.Bass()
    idx = nc.declare_dram_parameter("idx", [flat], mybir.dt.int32, isOutput=False)
    table = nc.declare_dram_parameter(
        "table", [vocab, d], mybir.dt.float32, isOutput=False
    )
    out = nc.declare_dram_parameter(
        "out", [flat, d], mybir.dt.float32, isOutput=True
    )

    idx_t = idx[:].rearrange("(n p k) -> p n k", p=P, k=k)        # [128, n, k]
    out_t = out[:].rearrange("(n p k) d -> n p (k d)", p=P, k=k)  # [n, 128, k*d]

    # One gather-sem and one store-sem PER SLOT: a shared counter would let
    # partial +1 increments from a later in-flight DMA satisfy an earlier
    # instruction's 16*(i+1) threshold (DMA completions interleave across
    # the 16 engines). Per-slot, at most one incrementer is in flight, so
    # every wait value is exact.
    with ExitStack() as ctx:
        it = ctx.enter_context(nc.sbuf_tensor([P, ntiles * k], mybir.dt.int32))
        ot = ctx.enter_context(
            nc.sbuf_tensor([P, bufs * k * d], mybir.dt.float32)
        )
        ls = ctx.enter_context(nc.semaphore("ls"))
        gsem = [ctx.enter_context(nc.semaphore(f"gs{s}")) for s in range(bufs)]
        ssem = [ctx.enter_context(nc.semaphore(f"ss{s}")) for s in range(bufs)]
        block = ctx.enter_context(nc.Block())

        @block.sync
        def _(sync):
            sync.dma_start(
                out=it[:].rearrange("p (n k) -> p n k", k=k), in_=idx_t
            ).then_inc(ls, 16)
            for i in range(ntiles):
                s, c = i % bufs, i // bufs
                # all k gathers of this group must have completed
                sync.wait_ge(gsem[s], 16 * k * (c + 1))
                sync.dma_start(
                    out=out_t[i], in_=ot[:, s * k * d : (s + 1) * k * d]
                ).then_inc(ssem[s], 16)

        @block.gpsimd
        def _(gpsimd):
            # HW descriptor generation consumes ONE index per partition per
            # indirect DMA (multi-index offset APs gather garbage beyond
            # col 0), so each group is k instructions of 128 rows each.
            gpsimd.wait_ge(ls, 16)
            for i in range(ntiles):
                s, c = i % bufs, i // bufs
                if c >= 1:
                    gpsimd.wait_ge(ssem[s], 16 * c)
                for j in range(k):
                    gpsimd.indirect_dma_start(
                        out=ot[:, (s * k + j) * d : (s * k + j + 1) * d],
                        out_offset=None,
                        in_=table[:, :],
                        in_offset=bass.IndirectOffsetOnAxis(
                            ap=it[:, i * k + j : i * k + j + 1], axis=0
                        ),
                    ).then_inc(gsem[s], 16)

    return nc


def run(indices, table, dummy=None, trace=False):
    global _built
    from concourse.bass_utils import run_bass_kernel_spmd

    if _built is None:
        _built = _build()
    nc = _built

    idx32 = np.ascontiguousarray(
        np.asarray(indices).reshape(NCORES, FLAT).astype(np.int32)
    )
    tab = np.ascontiguousarray(np.asarray(table), dtype=np.float32)
    in_maps = [{"idx": idx32[c], "table": tab} for c in range(NCORES)]
    kres = run_bass_kernel_spmd(nc, in_maps, list(range(NCORES)), trace=trace)
    out = np.concatenate(
        [kres.results[c]["out"].reshape(ROWS_PER_CORE, HIST, D) for c in range(NCORES)],
        axis=0,
    )
    return out, kres


def kernel(indices, table, dummy=None):
    return run(indices, table, dummy)[0]



# revision 3
# speedup vs baseline: 1.0729x; 1.0729x over previous
"""KVEmbedding lookup v2: host-sorted indices + InstDMAGatherAnt.

Strategy (the sharding_hint's all-to-all routing, with the host as the
network): indices are batch-sharded 8 ways; each core's 409600 lookups are
sorted by value on the host and bucketed into 31 windows of 32768 table rows
(int16 local offsets, the dma_gather addressing reach).  The device runs
14-sub-gather batches (1024 idxs each — the HW per-instruction limit) per
window: ~434 dma_gather instructions/core instead of 3200 tiny 128-row
indirect DMAs, so the ~1us/instruction SWDGE overhead drops 7x and the random
256B HBM reads become ascending (row-buffer friendly).  Gathered rows land in
sorted order; the host inverts the permutation during unshard.

Device per core: per window: 14x dma_gather(table[s*32K:(s+1)*32K]) -> SBUF
stage slot -> contiguous store to DRAM.  Double-buffered stages; gathers on
the Pool SWDGE queue, idx load + stores on the sync HWDGE queue.
"""

from contextlib import ExitStack

import numpy as np

BATCH, HIST = 16384, 200
VOCAB, D = 1_000_000, 64
NCORES = 8
ROWS_PER_CORE = BATCH // NCORES          # 2048
FLAT = ROWS_PER_CORE * HIST              # 409600 lookups per core
P = 128
SEG = 32768                               # int16-addressable window
NSEG = (VOCAB + SEG - 1) // SEG           # 31
SUB = 1024                                # max idxs per dma_gather on HW
CAP = 14336                               # slots per (core, window) bucket
NSUB = CAP // SUB                         # 14 sub-gathers per window
SLOTS = CAP // P                          # stage cols per window (112)

_built = {}


def _build(cap):
    import concourse.bacc as bacc
    import concourse.mybir as mybir
    from concourse._compat import get_trn_type

    nsub, slots = cap // SUB, cap // P
    sub_slots = SUB // P                  # 8 stage cols per sub-gather
    cw = cap // 16                        # idx16 cols per window
    nc = bacc.Bacc(
        get_trn_type() or "TRN2", num_swdge_queues=4,
        dynamic_dma_scratch_size=65536,
    )
    table = nc.declare_dram_parameter(
        "table", [VOCAB, D], mybir.dt.float32, isOutput=False
    )
    idx16 = nc.declare_dram_parameter(
        "idx16", [P, NSEG * cw], mybir.dt.int16, isOutput=False
    )
    out = nc.declare_dram_parameter(
        "out", [NSEG * cap, D], mybir.dt.float32, isOutput=True
    )

    with ExitStack() as ctx:
        t_idx = ctx.enter_context(
            nc.sbuf_tensor("t_idx", [P, NSEG * cw], mybir.dt.int16)
        )
        stage = [
            ctx.enter_context(
                nc.sbuf_tensor(f"stage{b}", [P, slots * D], mybir.dt.float32)
            )
            for b in range(2)
        ]
        ls = ctx.enter_context(nc.semaphore("ls"))
        gsem = [ctx.enter_context(nc.semaphore(f"gs{b}")) for b in range(2)]
        ssem = [ctx.enter_context(nc.semaphore(f"ss{b}")) for b in range(2)]
        block = ctx.enter_context(nc.Block())

        @block.sync
        def _(sync):
            sync.dma_start(out=t_idx[:], in_=idx16[:, :]).then_inc(ls, 16)
            for s in range(NSEG):
                b, c = s % 2, s // 2
                sync.wait_ge(gsem[b], 16 * nsub * (c + 1))
                sync.dma_start(
                    out=out[s * cap : (s + 1) * cap, :].rearrange(
                        "(p c) d -> p (c d)", p=P
                    ),
                    in_=stage[b][:],
                ).then_inc(ssem[b], 16)

        @block.gpsimd
        def _(gpsimd):
            from concourse import library_config

            gpsimd.load_library(library_config.mlp)
            gpsimd.wait_ge(ls, 16)
            rsub = gpsimd.to_reg(SUB)
            for s in range(NSEG):
                b, c = s % 2, s // 2
                if c >= 1:
                    gpsimd.wait_ge(ssem[b], 16 * c)
                lo = s * SEG
                hi = min((s + 1) * SEG, VOCAB)
                for j in range(nsub):
                    gpsimd.dma_gather(
                        out_ap=stage[b][
                            :, j * sub_slots * D : (j + 1) * sub_slots * D
                        ].rearrange("p (s d) -> p s d", s=sub_slots, d=D),
                        in_ap=table[lo:hi, :],
                        idxs_ap=t_idx[
                            :, s * cw + j * (SUB // 16) : s * cw + (j + 1) * (SUB // 16)
                        ],
                        num_idxs=SUB,
                        num_idxs_reg=rsub,
                        elem_size=D,
                        queue_num=j % 4,
                    ).then_inc(gsem[b], 16)

    nc.compile()
    return nc


def _pack_core(vals, cap):
    """Sort one core's lookups, bucket into NSEG int16 windows of `cap` slots.

    Returns (idx16 [P, NSEG*cap//16] int16, devrow [FLAT]: for each original
    position, the device output row holding its embedding)."""
    order = np.argsort(vals, kind="stable")
    sv = vals[order]
    bounds = np.searchsorted(sv, np.arange(NSEG + 1) * SEG)
    counts = np.diff(bounds)
    if counts.max() > cap:
        raise OverflowError(int(counts.max()))
    locals16 = (sv & (SEG - 1)).astype(np.int16)
    packed = np.zeros((NSEG, cap), np.int16)  # filler 0 = row s*SEG, harmless
    win = np.arange(FLAT) - bounds[:-1].repeat(counts)  # within-window pos
    seg_of = np.repeat(np.arange(NSEG), counts)
    packed[seg_of, win] = locals16
    # idx i of each SUB-chunk -> [i%16, i//16], replicated across the 8
    # partition groups
    blk = packed.reshape(NSEG * (cap // SUB), SUB // 16, 16)
    idx16 = np.tile(
        blk.transpose(2, 0, 1).reshape(1, 16, -1), (8, 1, 1)
    ).reshape(P, -1)
    # window pos w = j*SUB + i: stage col = j*(SUB//P) + i//P, partition i%P;
    # store is partition-major: device row = s*cap + part*slots + col
    slots = cap // P
    j, i = win // SUB, win % SUB
    devrow_sorted = seg_of * cap + (i % P) * slots + j * (SUB // P) + i // P
    devrow = np.empty(FLAT, np.int64)
    devrow[order] = devrow_sorted
    return idx16, devrow


def run(indices, table, dummy=None, trace=False, cap=CAP):
    from concourse.bass_utils import run_bass_kernel_spmd

    flat = np.asarray(indices).reshape(NCORES, FLAT).astype(np.int32)
    tab = np.ascontiguousarray(np.asarray(table), dtype=np.float32)

    while True:
        try:
            packs = [_pack_core(flat[c], cap) for c in range(NCORES)]
            break
        except OverflowError as e:
            cap = ((e.args[0] + SUB - 1) // SUB + 1) * SUB  # round up, retry

    if cap not in _built:
        _built[cap] = _build(cap)
    nc = _built[cap]

    in_maps = [{"idx16": packs[c][0], "table": tab} for c in range(NCORES)]
    kres = run_bass_kernel_spmd(nc, in_maps, list(range(NCORES)), trace=trace)
    out = np.empty((NCORES, FLAT, D), np.float32)
    for c in range(NCORES):
        dev = kres.results[c]["out"]
        out[c] = dev[packs[c][1]]
    return out.reshape(BATCH, HIST, D), kres


def kernel(indices, table, dummy=None):
    return run(indices, table, dummy)[0]
